# revision 1
# baseline (speedup 1.0000x reference)
import numpy as np

# Hardcoded problem configuration (nn_GaussianRenderer):
#   16384 gaussians, 512x512 image, 16px tiles -> 32x32 = 1024 tiles, K=64 per tile.
N_GAUSS = 16384
IMG_W = 512
IMG_H = 512
TILE = 16
K_MAX = 64


def _render(pos2d, cov2d, opacity, color, depth, width, height, t, K):
    Tx = width // t
    Ty = height // t
    T = Tx * Ty

    pos2d = np.asarray(pos2d, np.float32)
    cov2d = np.asarray(cov2d, np.float32)
    opacity = np.asarray(opacity, np.float32)
    color = np.asarray(color, np.float32)
    depth = np.asarray(depth, np.float32)

    # radius = 3 * sqrt(max eigenvalue of 2x2 covariance)
    a = cov2d[:, 0, 0]; b = cov2d[:, 0, 1]; c = cov2d[:, 1, 1]
    trace = a + c
    det = a * c - b * b
    term1 = 0.5 * trace
    term2 = 0.5 * np.sqrt(np.clip(trace * trace - 4.0 * det, 0.0, None))
    radius = 3.0 * np.sqrt(np.maximum(term1 - term2, term1 + term2))

    # global front-to-back depth sort (stable, matching jnp.argsort)
    order = np.argsort(depth, kind='stable')
    pos2d = pos2d[order]; cov2d = cov2d[order]
    opacity = opacity[order]; color = color[order]; radius = radius[order]

    # tile layout: tid = tx*Ty + ty; x runs along first image axis
    lefts = np.repeat(np.arange(Tx) * t, Ty).astype(np.float32)   # [T]
    tops = np.tile(np.arange(Ty) * t, Tx).astype(np.float32)      # [T]
    px = pos2d[None, :, 0]; py = pos2d[None, :, 1]; r = radius[None, :]
    L = lefts[:, None]; Tp = tops[:, None]
    overlap = (px + r > L) & (px - r < L + t) & (py + r > Tp) & (py - r < Tp + t)  # [T, N]

    # first K overlapping gaussians per tile, preserving depth order.
    # rank[i,j] = number of overlaps in tile i among gaussians 0..j; the
    # first K overlapping columns are exactly those with overlap & rank<=K.
    rank = np.cumsum(overlap, axis=1, dtype=np.int32)              # [T, N]
    counts = np.minimum(rank[:, -1], K)                            # [T]
    mask = overlap & (rank <= K)
    rows, cols = np.nonzero(mask)                                  # row-major => depth order
    slot = rank[rows, cols] - 1                                    # position within tile
    sel = np.zeros((T, K), dtype=np.int64)
    sel[rows, slot] = cols
    valid = np.arange(K)[None, :] < counts[:, None]                # [T, K]
    tp = pos2d[sel]          # [T, K, 2]
    tcov = cov2d[sel]        # [T, K, 2, 2]
    topac = opacity[sel]     # [T, K]
    tcol = color[sel]        # [T, K, 3]

    # per-tile pixel grid [T, t, t, 2], 'ij' indexing
    gi, gj = np.meshgrid(np.arange(t), np.arange(t), indexing='ij')
    base = np.stack([gi, gj], axis=-1).astype(np.float32)          # [t, t, 2]
    offs = np.stack([lefts, tops], axis=-1)                        # [T, 2]
    pix = base[None] + offs[:, None, None, :]                      # [T, t, t, 2]

    dx = pix[:, :, :, None, 0] - tp[:, None, None, :, 0]           # [T, t, t, K]
    dy = pix[:, :, :, None, 1] - tp[:, None, None, :, 1]
    ga = tcov[:, :, 0, 0][:, None, None, :]
    gb = tcov[:, :, 0, 1][:, None, None, :]
    gc = tcov[:, :, 1, 1][:, None, None, :]
    gdet = ga * gc - gb * gb
    quad = gc * dx * dx
    tmp = gb * dx
    tmp *= dy
    quad -= tmp
    quad -= tmp
    tmp = ga * dy
    tmp *= dy
    quad += tmp
    quad /= gdet
    quad *= np.float32(-0.5)
    prob = np.exp(quad, out=quad)                                  # [T, t, t, K]

    alpha = prob
    alpha *= topac[:, None, None, :]
    np.maximum(alpha, np.float32(0.01), out=alpha)
    np.minimum(alpha, np.float32(0.99), out=alpha)
    alpha *= valid[:, None, None, :]
    # transmittance: cumprod of (1 - alpha) shifted right by one, starting at 1
    weight = np.empty_like(alpha)
    weight[..., 0] = 1.0
    np.subtract(np.float32(1.0), alpha[..., :-1], out=weight[..., 1:])
    np.cumprod(weight, axis=-1, out=weight)
    weight *= alpha
    aw = weight.reshape(T, t * t, K)
    tile_img = np.matmul(aw, tcol).reshape(T, t, t, 3)             # [T, t, t, 3]

    img = tile_img.reshape(Tx, Ty, t, t, 3).transpose(0, 2, 1, 3, 4).reshape(width, height, 3)
    return img.astype(np.float32)


def kernel(pos2d, cov2d, opacity, color, depth, width=IMG_W, height=IMG_H,
           tile_length=TILE, max_per_tile=K_MAX):
    return _render(pos2d, cov2d, opacity, color, depth,
                   int(width), int(height), int(tile_length), int(max_per_tile))



# revision 10
# speedup vs baseline: 20.6577x; 20.6577x over previous
"""Gaussian tile rasterizer on 8 Trainium2 NeuronCores (Bass/Tile).

Problem config (hardcoded): 16384 gaussians, 512x512x3 image, 16px tiles
-> 1024 tiles, K=64 gaussians/tile, fp32.

Sharding: tile axis across 8 cores (128 tiles = 64 image rows per core);
gaussian arrays are reduced host-side to per-tile packed parameters.

Device math per tile (k = depth-ordered slot, p = local pixel):
  q'(k,p) = -0.5*quad + ln(opac)   as a rank-6 matmul  W[6,k]^T @ U[6,p]
            (U = [x^2, x, y^2, y, x*y, 1] over the 16x16 local grid; W folds
             the covariance inverse, local mean, opacity, and -0.5 scale;
             tf32 hi/lo split for fp32 accuracy at fp32r matmul speed)
  alpha   = exp(clip(q', ln .01, ln .99))          (clip in log domain)
  l       = ln(1 - alpha)
  lw      = Tri^T @ l      (strict-lower-triangular matmul = prefix sum)
  aw      = alpha * exp(lw)
  color   = aw^T @ tcol    (tcol zeroed on invalid slots, which makes the
            alpha valid-mask unnecessary: invalid slots only trail valid ones)
"""
import os
import sys
import time

import numpy as np

if '/opt/trn_rl_repo' not in sys.path:
    sys.path.insert(0, '/opt/trn_rl_repo')

N_GAUSS = 16384
IMG_W = 512
IMG_H = 512
T_LEN = 16
K_MAX = 64
Tx = IMG_W // T_LEN
Ty = IMG_H // T_LEN
T_TILES = Tx * Ty                       # 1024
N_CORES = 8
TILES_PER_CORE = T_TILES // N_CORES     # 128
PACKS = TILES_PER_CORE // 2             # 64 packs of 2 tiles
SUPER = PACKS // 2                      # 32 super-packs of 4 tiles

LOG_LO = float(np.log(np.float32(0.01)))
LOG_HI = float(np.log(np.float32(0.99)))


# ---------------------------------------------------------------- host prep
def _select(pos2d, cov2d, opacity, color, depth):
    """Depth-sort + per-tile first-K selection (identical to reference)."""
    a = cov2d[:, 0, 0]; b = cov2d[:, 0, 1]; c = cov2d[:, 1, 1]
    trace = a + c
    det = a * c - b * b
    term1 = np.float32(0.5) * trace
    term2 = np.float32(0.5) * np.sqrt(np.clip(trace * trace - np.float32(4.0) * det,
                                              np.float32(0.0), None))
    radius = np.float32(3.0) * np.sqrt(np.maximum(term1 - term2, term1 + term2))

    order = np.argsort(depth, kind='stable')
    pos2d = pos2d[order]; cov2d = cov2d[order]
    opacity = opacity[order]; color = color[order]; radius = radius[order]

    px = pos2d[:, 0]; py = pos2d[:, 1]; r = radius
    tx_lo = np.maximum(((px - r) * np.float32(1.0 / T_LEN)).astype(np.int32) - 1, 0)
    tx_hi = np.minimum(((px + r) * np.float32(1.0 / T_LEN)).astype(np.int32) + 1, Tx - 1)
    ty_lo = np.maximum(((py - r) * np.float32(1.0 / T_LEN)).astype(np.int32) - 1, 0)
    ty_hi = np.minimum(((py + r) * np.float32(1.0 / T_LEN)).astype(np.int32) + 1, Ty - 1)
    nxm = int((tx_hi - tx_lo).max()) + 1
    nym = int((ty_hi - ty_lo).max()) + 1

    gx = tx_lo[:, None] + np.arange(nxm, dtype=np.int32)[None, :]
    gy = ty_lo[:, None] + np.arange(nym, dtype=np.int32)[None, :]
    Lf = (gx << 4).astype(np.float32)
    Tf = (gy << 4).astype(np.float32)
    okx = (gx <= tx_hi[:, None]) \
        & (px[:, None] + r[:, None] > Lf) & (px[:, None] - r[:, None] < Lf + T_LEN)
    oky = (gy <= ty_hi[:, None]) \
        & (py[:, None] + r[:, None] > Tf) & (py[:, None] - r[:, None] < Tf + T_LEN)

    ok = okx[:, :, None] & oky[:, None, :]
    tid = gx[:, :, None].astype(np.int64) * Ty + gy[:, None, :]
    gidx, ii, jj = np.nonzero(ok)
    tids = tid[gidx, ii, jj]
    perm = np.argsort(tids, kind='stable')   # keeps depth order within tile
    tids_s = tids[perm]; g_s = gidx[perm]
    counts_full = np.bincount(tids_s, minlength=T_TILES)
    offs = np.zeros(T_TILES + 1, dtype=np.int64)
    np.cumsum(counts_full, out=offs[1:])
    slot = np.arange(tids_s.shape[0], dtype=np.int64) - offs[tids_s]
    keep = slot < K_MAX
    sel = np.zeros((T_TILES, K_MAX), dtype=np.int64)
    sel[tids_s[keep], slot[keep]] = g_s[keep]
    counts = np.minimum(counts_full, K_MAX)
    valid = np.arange(K_MAX)[None, :] < counts[:, None]
    return pos2d, cov2d, opacity, color, sel, valid


def _host_prep(pos2d, cov2d, opacity, color, depth):
    pos2d, cov2d, opacity, color, sel, valid = _select(
        pos2d, cov2d, opacity, color, depth)

    lefts = np.repeat(np.arange(Tx, dtype=np.float32) * T_LEN, Ty)
    tops = np.tile(np.arange(Ty, dtype=np.float32) * T_LEN, Tx)

    covs = cov2d[sel]
    ga = covs[:, :, 0, 0]; gb = covs[:, :, 0, 1]; gc = covs[:, :, 1, 1]
    inv = np.float32(1.0) / (ga * gc - gb * gb)
    A = gc * inv; B = gb * inv; C = ga * inv
    ps = pos2d[sel]
    mx = ps[:, :, 0] - lefts[:, None]
    my = ps[:, :, 1] - tops[:, None]
    lno = np.log(np.maximum(opacity[sel], np.float32(1e-30)))

    W = np.empty((T_TILES, 6, K_MAX), np.float32)
    W[:, 0] = A
    W[:, 1] = np.float32(-2.0) * A * mx + np.float32(2.0) * B * my
    W[:, 2] = C
    W[:, 3] = np.float32(-2.0) * C * my + np.float32(2.0) * B * mx
    W[:, 4] = np.float32(-2.0) * B
    W[:, 5] = A * mx * mx + C * my * my - np.float32(2.0) * B * mx * my \
        - np.float32(2.0) * lno
    W *= np.float32(-0.5)
    bad = ~valid
    W[:, 5][bad] = np.float32(-50.0)
    W[:, :5] *= valid[:, None, :]

    tcol = color[sel]
    tcol[bad] = 0.0

    per_core = []
    for cid in range(N_CORES):
        t0 = cid * TILES_PER_CORE
        w = W[t0:t0 + TILES_PER_CORE].transpose(1, 0, 2).reshape(
            6, TILES_PER_CORE * K_MAX)
        # compact color array [64, PACKS*2*3]: (k, (m, half, c)); device
        # scatters it into the block-diagonal [128, PACKS*6] layout
        tca = np.ascontiguousarray(
            tcol[t0:t0 + TILES_PER_CORE].reshape(PACKS, 2, K_MAX, 3)
            .transpose(2, 0, 1, 3).reshape(K_MAX, PACKS * 6))
        per_core.append({"w": np.ascontiguousarray(w), "tca": tca})
    return per_core


def _u_basis():
    i = np.arange(T_LEN, dtype=np.float32)
    gi, gj = np.meshgrid(i, i, indexing='ij')
    x = gi.ravel(); y = gj.ravel()
    return np.ascontiguousarray(
        np.stack([x * x, x, y * y, y, x * y, np.ones_like(x)], axis=0))


def _tri_basis():
    t64 = np.triu(np.ones((K_MAX, K_MAX), np.float32), 1)   # [j,k]: j<k
    tri = np.zeros((2 * K_MAX, 2 * K_MAX), np.float32)
    tri[:K_MAX, :K_MAX] = t64
    tri[K_MAX:, K_MAX:] = t64
    return tri


# ---------------------------------------------------------------- device code
_CACHE = {}


def _split_multiwait(nc, mybir):
    """Walrus in this toolchain rejects >1 sync wait on Drain instructions;
    split extra waits onto single-wait NoOps executed just before."""
    f = nc.m.functions[0]
    for bb in f.blocks:
        insts = list(bb.instructions)
        changed = False
        out = []
        for inst in insts:
            si = inst.sync_info
            if si is not None and si.on_wait is not None and len(si.on_wait) > 1:
                waits = list(si.on_wait)
                for i, w in enumerate(waits[:-1]):
                    out.append(mybir.InstNoOp(
                        name=f"{inst.name}_waitsplit{i}",
                        engine=inst.engine,
                        sync_info=mybir.SyncInfo(on_wait=[w], on_update=[]),
                    ))
                si.on_wait = [waits[-1]]
                changed = True
            out.append(inst)
        if changed:
            bb.instructions = out


def _build_nc():
    import concourse.bass as bass
    import concourse.mybir as mybir
    from concourse.tile import TileContext

    f32 = mybir.dt.float32
    f32r = mybir.dt.float32r
    AF = mybir.ActivationFunctionType
    OP = mybir.AluOpType

    nc = bass.Bass()
    w_d = nc.dram_tensor("w", [6, PACKS * 128], f32, kind="ExternalInput")
    tca_d = nc.dram_tensor("tca", [K_MAX, PACKS * 6], f32, kind="ExternalInput")
    tri_d = nc.dram_tensor("tri", [128, 128], f32r, kind="ExternalInput")
    uc_d = nc.dram_tensor("uc", [6, 256], f32, kind="ExternalInput")
    # raw block dump [blk=(txl,h), pixel(i8,j16), (ty,c)]; host reassembles
    img_d = nc.dram_tensor("img", [8, 128, 96], f32, kind="ExternalOutput")

    with TileContext(nc) as tc:
        with (
            tc.tile_pool(name="const", bufs=1) as cpool,
            tc.tile_pool(name="work", bufs=3) as wpool,
            tc.tile_pool(name="qps", bufs=2, space="PSUM") as qpool,
            tc.tile_pool(name="lws", bufs=2, space="PSUM") as lwpool,
            tc.tile_pool(name="cols", bufs=1, space="PSUM") as colpool,
        ):
            w_s = cpool.tile([6, PACKS * 128], f32, tag="w")
            tcols_s = cpool.tile([128, PACKS * 6], f32, tag="tcols")
            tri_s = cpool.tile([128, 128], f32r, tag="tri")
            uc_s = cpool.tile([6, 256], f32, tag="uc")
            nc.sync.dma_start(out=w_s[:], in_=w_d[:])
            nc.vector.memset(tcols_s[:], 0.0)
            tca_v = tca_d.rearrange("k (m h c) -> h k m c", m=PACKS, h=2, c=3)
            tcv = tcols_s[:].rearrange("(h k) (m hh c) -> h k m hh c", h=2, m=PACKS, hh=2, c=3)
            nc.sync.dma_start(out=tcv[0, :, :, 0, :], in_=tca_v[0])
            nc.sync.dma_start(out=tcv[1, :, :, 1, :], in_=tca_v[1])
            nc.sync.dma_start(out=tri_s[:], in_=tri_d[:])
            nc.sync.dma_start(out=uc_s[:], in_=uc_d[:])

            colp = colpool.tile([128, 1024], f32, tag="colp")

            for sp in range(SUPER):
                qp = qpool.tile([128, 512], f32, tag="qp")
                for half in range(2):
                    m = 2 * sp + half
                    out_ap = qp[:, half * 256:(half + 1) * 256]
                    nc.tensor.matmul(
                        out_ap,
                        lhsT=w_s[:, m * 128:(m + 1) * 128],
                        rhs=uc_s[:],
                        start=True, stop=True)
                qc = wpool.tile([128, 512], f32, tag="qc")
                nc.vector.tensor_scalar(qc[:], qp[:], LOG_HI, LOG_LO,
                                        op0=OP.min, op1=OP.max)
                al = wpool.tile([128, 512], f32, tag="al")
                nc.scalar.activation(al[:], qc[:], AF.Exp)
                ll = wpool.tile([128, 512], f32r, tag="ll")
                nc.scalar.activation(ll[:], al[:], AF.Ln, bias=1.0, scale=-1.0)
                lwp = lwpool.tile([128, 512], f32, tag="lwp")
                for half in range(2):
                    s = half * 256
                    nc.tensor.matmul(
                        lwp[:, s:s + 256],
                        lhsT=tri_s[:],
                        rhs=ll[:, s:s + 256],
                        start=True, stop=True)
                ww = wpool.tile([128, 512], f32, tag="ww")
                nc.scalar.activation(ww[:], lwp[:], AF.Exp)
                aw = wpool.tile([128, 512], f32, tag="aw")
                nc.vector.tensor_mul(aw[:], al[:], ww[:])
                for half in range(2):
                    m = 2 * sp + half
                    txl = (2 * m) // 32
                    ty0 = (2 * m) % 32
                    for h in range(2):
                        off = (txl * 2 + h) * 128 + ty0 * 3
                        nc.tensor.matmul(
                            colp[:, off:off + 6],
                            lhsT=aw[:, half * 256 + h * 128:
                                    half * 256 + (h + 1) * 128],
                            rhs=tcols_s[:, m * 6:(m + 1) * 6],
                            start=True, stop=True)

            colsb = cpool.tile([128, 1024], f32, tag="colsb")
            for txl in range(4):
                for h in range(2):
                    blk = txl * 2 + h
                    off = blk * 128
                    nc.vector.tensor_copy(colsb[:, off:off + 96],
                                          colp[:, off:off + 96])
                    nc.sync.dma_start(out=img_d[blk],
                                      in_=colsb[:, off:off + 96])

    _split_multiwait(nc, mybir)
    return nc


def _get_runtime():
    """Build the Bass module once and a cached jitted SPMD runner.

    First call goes through bass_utils.run_bass_kernel_spmd (the standard
    entry; compiles the NEFF). Subsequent calls reuse a jitted shard_map
    callable (same lowering) to skip per-call retracing, keep the constant
    tensors device-resident, and recycle the donated output buffer.
    """
    if "rt" in _CACHE:
        return _CACHE["rt"]

    import jax
    cache_dir = os.environ.get("BASS_JAX_CACHE_DIR",
                               os.path.expanduser("~/.cache/jax_bass_cache"))
    try:
        os.makedirs(cache_dir, exist_ok=True)
        jax.config.update("jax_compilation_cache_dir", cache_dir)
        jax.config.update("jax_persistent_cache_min_entry_size_bytes", -1)
        jax.config.update("jax_persistent_cache_min_compile_time_secs", 0.0)
    except Exception:
        pass

    import concourse.mybir as mybir
    from concourse import bass2jax
    from concourse.bass_utils import run_bass_kernel_spmd
    from jax.sharding import Mesh, PartitionSpec, NamedSharding

    nc = _build_nc()

    partition_name = (nc.partition_id_tensor.name
                      if nc.partition_id_tensor else None)
    in_names, out_names, out_avals, zero_shapes = [], [], [], []
    for alloc in nc.m.functions[0].allocations:
        if not isinstance(alloc, mybir.MemoryLocationSet):
            continue
        name = alloc.memorylocations[0].name
        if alloc.kind == "ExternalInput":
            if name != partition_name:
                in_names.append(name)
        elif alloc.kind == "ExternalOutput":
            shape = tuple(alloc.tensor_shape)
            dtype = mybir.dt.np(alloc.dtype)
            out_names.append(name)
            out_avals.append(jax.core.ShapedArray(shape, dtype))
            zero_shapes.append((shape, dtype))
    n_params = len(in_names)
    n_outs = len(out_avals)
    all_in_names = list(in_names) + list(out_names) \
        + ([partition_name] if partition_name else [])
    donate = tuple(range(n_params, n_params + n_outs))

    def _body(*args):
        operands = list(args)
        if partition_name is not None:
            operands.append(bass2jax.partition_id_tensor())
        outs = bass2jax._bass_exec_p.bind(
            *operands, out_avals=tuple(out_avals), in_names=tuple(all_in_names),
            out_names=tuple(out_names), lowering_input_output_aliases=(),
            sim_require_finite=True, sim_require_nnan=True, nc=nc)
        return tuple(outs)

    devices = jax.devices()[:N_CORES]
    mesh = Mesh(np.asarray(devices), ("core",))
    in_specs = (PartitionSpec("core"),) * (n_params + n_outs)
    out_specs = (PartitionSpec("core"),) * n_outs
    sharded = jax.jit(
        bass2jax.shard_map(_body, mesh=mesh, in_specs=in_specs,
                           out_specs=out_specs, check_rep=False),
        donate_argnums=donate, keep_unused=True)
    shardspec = NamedSharding(mesh, PartitionSpec("core"))

    state = {"first": True, "prev_out": None, "const_dev": None}

    def run(per_core, uc, tri):
        if state["first"]:
            state["first"] = False
            in_maps = [{**per_core[c], "tri": tri, "uc": uc}
                       for c in range(N_CORES)]
            res = run_bass_kernel_spmd(nc, in_maps, list(range(N_CORES)))
            return [res.results[c]["img"] for c in range(N_CORES)]

        if state["const_dev"] is None:
            state["const_dev"] = {
                "tri": jax.device_put(
                    np.concatenate([tri] * N_CORES, axis=0), shardspec),
                "uc": jax.device_put(
                    np.concatenate([uc] * N_CORES, axis=0), shardspec),
            }
        cd = state["const_dev"]
        concat = {n: np.concatenate([per_core[c][n] for c in range(N_CORES)],
                                    axis=0)
                  for n in per_core[0]}
        args = []
        for n in in_names:
            args.append(cd[n] if n in cd else concat[n])
        # donated output operand: recycle last call's output buffer (the
        # kernel overwrites every element, so contents are irrelevant)
        prev = state["prev_out"]
        for i, (s, d) in enumerate(zero_shapes):
            if prev is not None and not prev[i].is_deleted():
                args.append(prev[i])
            else:
                args.append(np.zeros((N_CORES * s[0], *s[1:]), d))
        out_arrs = sharded(*args)
        state["prev_out"] = list(out_arrs)
        return [np.asarray(out_arrs[0]).reshape(N_CORES, *out_avals[0].shape)[c]
                for c in range(N_CORES)]

    _CACHE["rt"] = run
    return run


# ---------------------------------------------------------------- entry point
def kernel(pos2d, cov2d, opacity, color, depth, width=IMG_W, height=IMG_H,
           tile_length=T_LEN, max_per_tile=K_MAX):
    assert int(width) == IMG_W and int(height) == IMG_H
    assert int(tile_length) == T_LEN and int(max_per_tile) == K_MAX

    pos2d = np.ascontiguousarray(pos2d, np.float32)
    cov2d = np.ascontiguousarray(cov2d, np.float32)
    opacity = np.ascontiguousarray(opacity, np.float32)
    color = np.ascontiguousarray(color, np.float32)
    depth = np.ascontiguousarray(depth, np.float32)

    run = _get_runtime()
    per_core = _host_prep(pos2d, cov2d, opacity, color, depth)
    uc = _u_basis()
    tri = _tri_basis()
    imgs = run(per_core, uc, tri)
    blocks = np.stack(imgs, axis=0)
    # [core, blk(txl,h), p(i8,j16), f(ty,c)] -> [512, 512, 3]
    img = blocks.reshape(N_CORES, 4, 2, 8, 16, 32, 3) \
        .transpose(0, 1, 2, 3, 5, 4, 6).reshape(IMG_W, IMG_H, 3)
    return np.ascontiguousarray(img, np.float32)


# revision 17
# speedup vs baseline: 22.4969x; 1.0890x over previous
"""Gaussian tile rasterizer on 8 Trainium2 NeuronCores (Bass/Tile).

Problem config (hardcoded): 16384 gaussians, 512x512x3 image, 16px tiles
-> 1024 tiles, K=64 gaussians/tile, fp32.

Sharding: tile axis across 8 cores (128 tiles = 64 image rows per core);
gaussian arrays are reduced host-side to per-tile packed parameters.

Device math per tile (k = depth-ordered slot, p = local pixel):
  q'(k,p) = -0.5*quad + ln(opac)   as a rank-6 matmul  W[6,k]^T @ U[6,p]
            (U = [x^2, x, y^2, y, x*y, 1] over the 16x16 local grid; W folds
             the covariance inverse, local mean, opacity, and -0.5 scale;
             tf32 hi/lo split for fp32 accuracy at fp32r matmul speed)
  alpha   = exp(clip(q', ln .01, ln .99))          (clip in log domain)
  l       = ln(1 - alpha)
  lw      = Tri^T @ l      (strict-lower-triangular matmul = prefix sum)
  aw      = alpha * exp(lw)
  color   = aw^T @ tcol    (tcol zeroed on invalid slots, which makes the
            alpha valid-mask unnecessary: invalid slots only trail valid ones)
"""
import os
import sys
import time

import numpy as np

if '/opt/trn_rl_repo' not in sys.path:
    sys.path.insert(0, '/opt/trn_rl_repo')

N_GAUSS = 16384
IMG_W = 512
IMG_H = 512
T_LEN = 16
K_MAX = 64
Tx = IMG_W // T_LEN
Ty = IMG_H // T_LEN
T_TILES = Tx * Ty                       # 1024
N_CORES = 8
TILES_PER_CORE = T_TILES // N_CORES     # 128
PACKS = TILES_PER_CORE // 2             # 64 packs of 2 tiles
SUPER = PACKS // 2                      # 32 super-packs of 4 tiles

LOG_LO = float(np.log(np.float32(0.01)))
LOG_HI = float(np.log(np.float32(0.99)))


# ---------------------------------------------------------------- host prep
def _select(pos2d, cov2d, opacity, color, depth):
    """Depth-sort + per-tile first-K selection (identical to reference)."""
    a = cov2d[:, 0, 0]; b = cov2d[:, 0, 1]; c = cov2d[:, 1, 1]
    trace = a + c
    det = a * c - b * b
    term1 = np.float32(0.5) * trace
    term2 = np.float32(0.5) * np.sqrt(np.clip(trace * trace - np.float32(4.0) * det,
                                              np.float32(0.0), None))
    radius = np.float32(3.0) * np.sqrt(np.maximum(term1 - term2, term1 + term2))

    order = np.argsort(depth, kind='stable')
    pos2d = pos2d[order]; cov2d = cov2d[order]
    opacity = opacity[order]; color = color[order]; radius = radius[order]

    px = pos2d[:, 0]; py = pos2d[:, 1]; r = radius
    tx_lo = np.maximum(((px - r) * np.float32(1.0 / T_LEN)).astype(np.int32) - 1, 0)
    tx_hi = np.minimum(((px + r) * np.float32(1.0 / T_LEN)).astype(np.int32) + 1, Tx - 1)
    ty_lo = np.maximum(((py - r) * np.float32(1.0 / T_LEN)).astype(np.int32) - 1, 0)
    ty_hi = np.minimum(((py + r) * np.float32(1.0 / T_LEN)).astype(np.int32) + 1, Ty - 1)
    nxm = int((tx_hi - tx_lo).max()) + 1
    nym = int((ty_hi - ty_lo).max()) + 1

    gx = tx_lo[:, None] + np.arange(nxm, dtype=np.int32)[None, :]
    gy = ty_lo[:, None] + np.arange(nym, dtype=np.int32)[None, :]
    Lf = (gx << 4).astype(np.float32)
    Tf = (gy << 4).astype(np.float32)
    okx = (gx <= tx_hi[:, None]) \
        & (px[:, None] + r[:, None] > Lf) & (px[:, None] - r[:, None] < Lf + T_LEN)
    oky = (gy <= ty_hi[:, None]) \
        & (py[:, None] + r[:, None] > Tf) & (py[:, None] - r[:, None] < Tf + T_LEN)

    ok = okx[:, :, None] & oky[:, None, :]
    tid = gx[:, :, None].astype(np.int64) * Ty + gy[:, None, :]
    gidx, ii, jj = np.nonzero(ok)
    tids = tid[gidx, ii, jj]
    perm = np.argsort(tids, kind='stable')   # keeps depth order within tile
    tids_s = tids[perm]; g_s = gidx[perm]
    counts_full = np.bincount(tids_s, minlength=T_TILES)
    offs = np.zeros(T_TILES + 1, dtype=np.int64)
    np.cumsum(counts_full, out=offs[1:])
    slot = np.arange(tids_s.shape[0], dtype=np.int64) - offs[tids_s]
    keep = slot < K_MAX
    sel = np.zeros((T_TILES, K_MAX), dtype=np.int64)
    sel[tids_s[keep], slot[keep]] = g_s[keep]
    counts = np.minimum(counts_full, K_MAX)
    valid = np.arange(K_MAX)[None, :] < counts[:, None]
    return pos2d, cov2d, opacity, color, sel, valid


def _host_prep(pos2d, cov2d, opacity, color, depth):
    pos2d, cov2d, opacity, color, sel, valid = _select(
        pos2d, cov2d, opacity, color, depth)

    lefts = np.repeat(np.arange(Tx, dtype=np.float32) * T_LEN, Ty)
    tops = np.tile(np.arange(Ty, dtype=np.float32) * T_LEN, Tx)

    covs = cov2d[sel]
    ga = covs[:, :, 0, 0]; gb = covs[:, :, 0, 1]; gc = covs[:, :, 1, 1]
    inv = np.float32(1.0) / (ga * gc - gb * gb)
    A = gc * inv; B = gb * inv; C = ga * inv
    ps = pos2d[sel]
    mx = ps[:, :, 0] - lefts[:, None]
    my = ps[:, :, 1] - tops[:, None]
    lno = np.log(np.maximum(opacity[sel], np.float32(1e-30)))

    W = np.empty((T_TILES, 6, K_MAX), np.float32)
    W[:, 0] = A
    W[:, 1] = np.float32(-2.0) * A * mx + np.float32(2.0) * B * my
    W[:, 2] = C
    W[:, 3] = np.float32(-2.0) * C * my + np.float32(2.0) * B * mx
    W[:, 4] = np.float32(-2.0) * B
    W[:, 5] = A * mx * mx + C * my * my - np.float32(2.0) * B * mx * my \
        - np.float32(2.0) * lno
    W *= np.float32(-0.5)
    bad = ~valid
    W[:, 5][bad] = np.float32(-50.0)
    W[:, :5] *= valid[:, None, :]

    tcol = color[sel]
    tcol[bad] = 0.0

    per_core = []
    for cid in range(N_CORES):
        t0 = cid * TILES_PER_CORE
        w = W[t0:t0 + TILES_PER_CORE].transpose(1, 0, 2).reshape(
            6, TILES_PER_CORE * K_MAX)
        # compact color array [64, PACKS*2*3]: (k, (m, half, c)); device
        # scatters it into the block-diagonal [128, PACKS*6] layout
        tca = np.ascontiguousarray(
            tcol[t0:t0 + TILES_PER_CORE].reshape(PACKS, 2, K_MAX, 3)
            .transpose(2, 0, 1, 3).reshape(K_MAX, PACKS * 6))
        per_core.append({"w": np.ascontiguousarray(w), "tca": tca})
    return per_core


def _u_basis():
    i = np.arange(T_LEN, dtype=np.float32)
    gi, gj = np.meshgrid(i, i, indexing='ij')
    x = gi.ravel(); y = gj.ravel()
    return np.ascontiguousarray(
        np.stack([x * x, x, y * y, y, x * y, np.ones_like(x)], axis=0))


def _tri_basis():
    t64 = np.triu(np.ones((K_MAX, K_MAX), np.float32), 1)   # [j,k]: j<k
    tri = np.zeros((2 * K_MAX, 2 * K_MAX), np.float32)
    tri[:K_MAX, :K_MAX] = t64
    tri[K_MAX:, K_MAX:] = t64
    return tri


# ---------------------------------------------------------------- device code
_CACHE = {}


def _split_multiwait(nc, mybir):
    """Walrus in this toolchain rejects >1 sync wait on Drain instructions;
    split extra waits onto single-wait NoOps executed just before."""
    f = nc.m.functions[0]
    for bb in f.blocks:
        insts = list(bb.instructions)
        changed = False
        out = []
        for inst in insts:
            si = inst.sync_info
            if si is not None and si.on_wait is not None and len(si.on_wait) > 1:
                waits = list(si.on_wait)
                for i, w in enumerate(waits[:-1]):
                    out.append(mybir.InstNoOp(
                        name=f"{inst.name}_waitsplit{i}",
                        engine=inst.engine,
                        sync_info=mybir.SyncInfo(on_wait=[w], on_update=[]),
                    ))
                si.on_wait = [waits[-1]]
                changed = True
            out.append(inst)
        if changed:
            bb.instructions = out


def _build_nc(split=True):
    import concourse.bass as bass
    import concourse.mybir as mybir
    from concourse.tile import TileContext

    f32 = mybir.dt.float32
    f32r = mybir.dt.float32r
    f16 = mybir.dt.float16
    AF = mybir.ActivationFunctionType
    OP = mybir.AluOpType

    nc = bass.Bass()
    w_d = nc.dram_tensor("w", [6, PACKS * 128], f32, kind="ExternalInput")
    tca_d = nc.dram_tensor("tca", [K_MAX, PACKS * 6], f32, kind="ExternalInput")
    tri_d = nc.dram_tensor("tri", [128, 128], f32r, kind="ExternalInput")
    uc_d = nc.dram_tensor("uc", [6, 256], f32, kind="ExternalInput")
    # raw block dump [blk=(txl,h), pixel(i8,j16), (ty,c)]; host reassembles
    img_d = nc.dram_tensor("img", [8, 128, 96], f32, kind="ExternalOutput")

    with TileContext(nc) as tc:
        with (
            tc.tile_pool(name="const", bufs=1) as cpool,
            tc.tile_pool(name="work", bufs=3) as wpool,
            tc.tile_pool(name="qps", bufs=2, space="PSUM") as qpool,
            tc.tile_pool(name="lws", bufs=2, space="PSUM") as lwpool,
            tc.tile_pool(name="cols", bufs=1, space="PSUM") as colpool,
        ):
            w_s = cpool.tile([6, PACKS * 128], f32, tag="w")
            tcols_s = cpool.tile([128, PACKS * 6], f32, tag="tcols")
            tri_s = cpool.tile([128, 128], f32r, tag="tri")
            uc_s = cpool.tile([6, 256], f32, tag="uc")
            nc.sync.dma_start(out=w_s[:], in_=w_d[:])
            nc.vector.memset(tcols_s[:], 0.0)
            tca_v = tca_d.rearrange("k (m h c) -> h k m c", m=PACKS, h=2, c=3)
            tcv = tcols_s[:].rearrange("(h k) (m hh c) -> h k m hh c", h=2, m=PACKS, hh=2, c=3)
            nc.sync.dma_start(out=tcv[0, :, :, 0, :], in_=tca_v[0])
            nc.sync.dma_start(out=tcv[1, :, :, 1, :], in_=tca_v[1])
            nc.sync.dma_start(out=tri_s[:], in_=tri_d[:])
            nc.sync.dma_start(out=uc_s[:], in_=uc_d[:])

            colp = colpool.tile([128, 1024], f32, tag="colp")

            for sp in range(SUPER):
                qp = qpool.tile([128, 512], f32, tag="qp")
                for half in range(2):
                    m = 2 * sp + half
                    out_ap = qp[:, half * 256:(half + 1) * 256]
                    nc.tensor.matmul(
                        out_ap,
                        lhsT=w_s[:, m * 128:(m + 1) * 128],
                        rhs=uc_s[:],
                        start=True, stop=True)
                qc = wpool.tile([128, 512], f32, tag="qc")
                nc.vector.tensor_scalar(qc[:], qp[:], LOG_HI, LOG_LO,
                                        op0=OP.min, op1=OP.max)
                al = wpool.tile([128, 512], f32, tag="al")
                nc.scalar.activation(al[:], qc[:], AF.Exp)
                ll = wpool.tile([128, 512], f32r, tag="ll")
                nc.scalar.activation(ll[:], al[:], AF.Ln, bias=1.0, scale=-1.0)
                lwp = lwpool.tile([128, 512], f32, tag="lwp")
                for half in range(2):
                    s = half * 256
                    nc.tensor.matmul(
                        lwp[:, s:s + 256],
                        lhsT=tri_s[:],
                        rhs=ll[:, s:s + 256],
                        start=True, stop=True)
                ww = wpool.tile([128, 512], f32, tag="ww")
                nc.scalar.activation(ww[:], lwp[:], AF.Exp)
                aw = wpool.tile([128, 512], f32, tag="aw")
                nc.vector.tensor_mul(aw[:], al[:], ww[:])
                for half in range(2):
                    m = 2 * sp + half
                    txl = (2 * m) // 32
                    ty0 = (2 * m) % 32
                    for h in range(2):
                        off = (txl * 2 + h) * 128 + ty0 * 3
                        nc.tensor.matmul(
                            colp[:, off:off + 6],
                            lhsT=aw[:, half * 256 + h * 128:
                                    half * 256 + (h + 1) * 128],
                            rhs=tcols_s[:, m * 6:(m + 1) * 6],
                            start=True, stop=True)

            colsb = cpool.tile([128, 1024], f32, tag="colsb")
            for txl in range(4):
                for h in range(2):
                    blk = txl * 2 + h
                    off = blk * 128
                    nc.vector.tensor_copy(colsb[:, off:off + 96],
                                          colp[:, off:off + 96])
                    nc.sync.dma_start(out=img_d[blk],
                                      in_=colsb[:, off:off + 96])

    if split:
        _split_multiwait(nc, mybir)
    return nc


def _get_runtime():
    """Build the Bass module once and a cached jitted SPMD runner.

    First call goes through bass_utils.run_bass_kernel_spmd (the standard
    entry; compiles the NEFF). Subsequent calls reuse a jitted shard_map
    callable (same lowering) to skip per-call retracing, keep the constant
    tensors device-resident, and recycle the donated output buffer.
    """
    if "rt" in _CACHE:
        return _CACHE["rt"]

    import jax
    cache_dir = os.environ.get("BASS_JAX_CACHE_DIR",
                               os.path.expanduser("~/.cache/jax_bass_cache"))
    try:
        os.makedirs(cache_dir, exist_ok=True)
        jax.config.update("jax_compilation_cache_dir", cache_dir)
        jax.config.update("jax_persistent_cache_min_entry_size_bytes", -1)
        jax.config.update("jax_persistent_cache_min_compile_time_secs", 0.0)
    except Exception:
        pass

    import concourse.mybir as mybir
    from concourse import bass2jax
    from concourse.bass_utils import run_bass_kernel_spmd
    from jax.sharding import Mesh, PartitionSpec, NamedSharding

    nc = _build_nc()

    partition_name = (nc.partition_id_tensor.name
                      if nc.partition_id_tensor else None)
    in_names, out_names, out_avals, zero_shapes = [], [], [], []
    for alloc in nc.m.functions[0].allocations:
        if not isinstance(alloc, mybir.MemoryLocationSet):
            continue
        name = alloc.memorylocations[0].name
        if alloc.kind == "ExternalInput":
            if name != partition_name:
                in_names.append(name)
        elif alloc.kind == "ExternalOutput":
            shape = tuple(alloc.tensor_shape)
            dtype = mybir.dt.np(alloc.dtype)
            out_names.append(name)
            out_avals.append(jax.core.ShapedArray(shape, dtype))
            zero_shapes.append((shape, dtype))
    n_params = len(in_names)
    n_outs = len(out_avals)
    all_in_names = list(in_names) + list(out_names) \
        + ([partition_name] if partition_name else [])
    donate = tuple(range(n_params, n_params + n_outs))

    def _body(*args):
        operands = list(args)
        if partition_name is not None:
            operands.append(bass2jax.partition_id_tensor())
        outs = bass2jax._bass_exec_p.bind(
            *operands, out_avals=tuple(out_avals), in_names=tuple(all_in_names),
            out_names=tuple(out_names), lowering_input_output_aliases=(),
            sim_require_finite=True, sim_require_nnan=True, nc=nc)
        return tuple(outs)

    devices = jax.devices()[:N_CORES]
    mesh = Mesh(np.asarray(devices), ("core",))
    in_specs = (PartitionSpec("core"),) * (n_params + n_outs)
    out_specs = (PartitionSpec("core"),) * n_outs
    sharded = jax.jit(
        bass2jax.shard_map(_body, mesh=mesh, in_specs=in_specs,
                           out_specs=out_specs, check_rep=False),
        donate_argnums=donate, keep_unused=True)
    shardspec = NamedSharding(mesh, PartitionSpec("core"))

    state = {"first": True, "prev_out": None, "const_dev": None}

    def run(per_core, uc, tri):
        if state["first"]:
            state["first"] = False
            in_maps = [{**per_core[c], "tri": tri, "uc": uc}
                       for c in range(N_CORES)]
            res = run_bass_kernel_spmd(nc, in_maps, list(range(N_CORES)))
            return [res.results[c]["img"] for c in range(N_CORES)]

        if state["const_dev"] is None:
            state["const_dev"] = {
                "tri": jax.device_put(
                    np.concatenate([tri] * N_CORES, axis=0), shardspec),
                "uc": jax.device_put(
                    np.concatenate([uc] * N_CORES, axis=0), shardspec),
            }
        cd = state["const_dev"]
        concat = {n: np.concatenate([per_core[c][n] for c in range(N_CORES)],
                                    axis=0)
                  for n in per_core[0]}
        args = []
        for n in in_names:
            args.append(cd[n] if n in cd else concat[n])
        # donated output operand: recycle last call's output buffer (the
        # kernel overwrites every element, so contents are irrelevant)
        prev = state["prev_out"]
        for i, (s, d) in enumerate(zero_shapes):
            if prev is not None and not prev[i].is_deleted():
                args.append(prev[i])
            else:
                args.append(jax.device_put(
                    np.zeros((N_CORES * s[0], *s[1:]), d), shardspec))
        out_arrs = sharded(*args)
        state["prev_out"] = list(out_arrs)
        return [np.asarray(out_arrs[0]).reshape(N_CORES, *out_avals[0].shape)[c]
                for c in range(N_CORES)]

    _CACHE["rt"] = run
    return run


# ---------------------------------------------------------------- entry point
def kernel(pos2d, cov2d, opacity, color, depth, width=IMG_W, height=IMG_H,
           tile_length=T_LEN, max_per_tile=K_MAX):
    assert int(width) == IMG_W and int(height) == IMG_H
    assert int(tile_length) == T_LEN and int(max_per_tile) == K_MAX

    pos2d = np.ascontiguousarray(pos2d, np.float32)
    cov2d = np.ascontiguousarray(cov2d, np.float32)
    opacity = np.ascontiguousarray(opacity, np.float32)
    color = np.ascontiguousarray(color, np.float32)
    depth = np.ascontiguousarray(depth, np.float32)

    run = _get_runtime()
    per_core = _host_prep(pos2d, cov2d, opacity, color, depth)
    uc = _u_basis()
    tri = _tri_basis()
    imgs = run(per_core, uc, tri)
    blocks = np.stack(imgs, axis=0)
    # [core, blk(txl,h), p(i8,j16), f(ty,c)] -> [512, 512, 3]
    img = blocks.reshape(N_CORES, 4, 2, 8, 16, 32, 3) \
        .transpose(0, 1, 2, 3, 5, 4, 6).reshape(IMG_W, IMG_H, 3)
    return np.ascontiguousarray(img, np.float32)
